# revision 114
# baseline (speedup 1.0000x reference)
"""Masked multi-head attention on 8 Trainium2 NeuronCores.

Sharding: core c = (b, hg) with b = c // 4, hg = c % 4. Each core computes the
full attention block for batch b restricted to heads [4*hg, 4*hg+4), including
its slice of the QKV projection and of the output projection. The host sums the
4 tensor-parallel partial outputs per batch and adds the output bias (which
also absorbs the V bias term: (O + bv) @ Wout = O @ Wout + bv @ Wout).

Shapes are hardcoded for B=2, T=2048, D=1024, H=16, Hd=64.

The data path is bf16 (x^T, weights, Q^T/K^T, P^T, V, O^T, out) with every
matmul accumulating in fp32 PSUM and the QK^T logits formed in fp32; only
input rounding is bf16. PE matmul throughput on TRN2 is identical for bf16
and fp32r, but bf16 halves DMA + SBUF and lifts the fp32r >=256 moving-dim
constraint, so diagonal score tiles trim to their true causal width.

Schedule highlights:
- host pre-transposes x, so the kernel runs zero PE transposes;
- Q/K bias fused into the PSUM evacuation (DVE tensor_scalar, per-partition);
- V bias folded into bout on the host;
- one paired exp per k-tile covers both heads of a pair (keys on partitions,
  queries on the free dim feed the AV matmul directly);
- softmax denominators ride a 65th ones-column in the V tiles; the normalize
  (reciprocal -> ones-broadcast matmul -> multiply) is deferred into the next
  pair's filler stream so it stays off the PE critical path, with its
  broadcast landing in the score (sp) PSUM slots to keep the pq slots free;
- scores run 5 k-tiles ahead of the AV consuming them, and independent PE
  work (next block's QKV, deferred out-projections) is popped from a
  cost-weighted filler queue so the PE is never gated on exp latency;
- filler tranches are balanced against each block's Act load: K columns and
  V rows of block i fill inside their own block (first consumed at k-tile
  4i), out-projections run two blocks late where QKV filler has run out;
- block 0's Q/K accumulates 4-ways in parallel (pq + the idle av banks)
  over dt-chunked wqk/x DMAs, behind a PE p-state warmup chain;
- the tail staggers output staging across Act + DVE and drains the last
  DMAs through both the Act HWDGE and Pool SWDGE queues in parallel.
"""

import numpy as np
from contextlib import ExitStack
from collections import deque

import ml_dtypes

import concourse.bass as bass
import concourse.bacc as bacc
import concourse.mybir as mybir
import concourse.tile as tile
from concourse.bass_utils import run_bass_kernel_spmd

B, T, D = 2, 2048, 1024
H, HD = 16, 64
HL = 4               # heads per core
NCORES = 8
TQ = 512             # query tile (matmul moving free dim)
TK = 128             # key tile
NQT = T // TQ        # 4
NKT = T // TK        # 16
NDT = D // 128       # 8

F32 = mybir.dt.float32
BF16 = mybir.dt.bfloat16
EXP = mybir.ActivationFunctionType.Exp
COPY = mybir.ActivationFunctionType.Copy
MULT = mybir.AluOpType.mult
ADD = mybir.AluOpType.add

LAST_RESULTS = None  # BassKernelResults of the most recent run (for test.py)


def _build_mha(tc, out_ap, in_aps):
    nc = tc.nc
    xt_d = in_aps["xt"]        # [D, T]  (x transposed on host, bf16)
    wqk_d = in_aps["wqk"]      # [D, 512]  (Qh0|Qh1|Qh2|Qh3|Kh0..Kh3, Q pre-scaled)
    bqk_d = in_aps["bqk"]      # [128, 4]  (col fb = per-partition bias, f32)
    wv_d = in_aps["wv"]        # [D, 256]
    wout_d = in_aps["wout"]    # [128, 2, 1024]
    mask_d = in_aps["mask"]    # [128, 896]
    sel2_d = in_aps["sel2"]    # [2, 128]: row0 = 64x1,64x0; row1 = 64x0,64x1

    with ExitStack() as ctx:
        ctx.enter_context(nc.allow_low_precision(reason="bf16 matmul pipeline"))
        const = ctx.enter_context(tc.tile_pool(name="const", bufs=1))
        big = ctx.enter_context(tc.tile_pool(name="big", bufs=1))

        # Persistent activations.
        # qkt[:, fb, t]: fb 0,1 = Q^T head pairs (0,1),(2,3); fb 2,3 = K^T pairs.
        # Rows 0:64 = even head of the pair, 64:128 = odd head.
        qkt = big.tile([128, 4, T], BF16)
        # vp[:, kt, h, 0:64] = V[kt*128:+128, h*64:+64]; vp[..., 64] = 1.0
        vp = big.tile([128, NKT, HL, 65], BF16)
        # ot[:, p, t]: normalized attention output^T; rows by head as in qkt
        ot = big.tile([128, 2, T], BF16)

        with ExitStack() as pctx:
            # PSUM: sp(2 banks x2) + pq(1 x2) + av0 + av1 = 8 banks
            ps = pctx.enter_context(tc.tile_pool(name="ps", bufs=2, space="PSUM"))
            ps_av = pctx.enter_context(tc.tile_pool(name="ps_av", bufs=1,
                                                    space="PSUM"))
            wpool = pctx.enter_context(tc.tile_pool(name="w", bufs=1))
            xt_p = pctx.enter_context(tc.tile_pool(name="xt", bufs=4))
            pt_p = pctx.enter_context(tc.tile_pool(name="ptile", bufs=4))
            nrm_p = pctx.enter_context(tc.tile_pool(name="nrm", bufs=2))

            # PE p-state warmup: harmless tiny matmuls keep the tensor engine
            # continuously busy through its ramp while the first DMAs land.
            # The operand comes from an on-device memset (no DMA round-trip).
            wsrc = const.tile([1, 64], BF16)
            nc.gpsimd.memset(wsrc[:], 1.0)
            warm = ps.tile([1, 64], F32, tag="pq")
            for _ in range(64):
                nc.tensor.matmul(warm[:], wsrc[0:1, 0:1], wsrc[:],
                                 start=True, stop=True)

            # softmax-denominator ones column of vp, built in-place
            nc.gpsimd.memset(vp[:, :, :, 64], 1.0)

            xts = {}

            def emit_xt_dma(tt):
                xt = xt_p.tile([128, NDT, TQ], BF16, tag="xt", name=f"xt_{tt}")
                src = xt_d[:, tt * TQ:(tt + 1) * TQ].rearrange(
                    "(o p) t -> p o t", p=128)
                nc.sync.dma_start(xt[:, 0:4, :], src[:, 0:4, :])
                nc.sync.dma_start(xt[:, 4:8, :], src[:, 4:8, :])
                xts[tt] = xt

            # DMA order = first-use order: the cost model's DMA engine pool
            # drains in issue order, so front-load what gates the pipeline.
            # wqk/xt0 stream in dt-pair chunks so the 4-way parallel QKV(0)
            # accumulation below can start on the first chunk.
            wqk = wpool.tile([128, NDT, 512], BF16)
            wqk_src = wqk_d.rearrange("(o p) f -> p o f", p=128)
            xt0 = xt_p.tile([128, NDT, TQ], BF16, tag="xt", name="xt_0")
            xt0_src = xt_d[:, 0:TQ].rearrange("(o p) t -> p o t", p=128)
            xts[0] = xt0
            for dh in range(4):
                s = slice(2 * dh, 2 * dh + 2)
                nc.sync.dma_start(wqk[:, s, :], wqk_src[:, s, :])
                nc.sync.dma_start(xt0[:, s, :], xt0_src[:, s, :])
            sel2 = const.tile([2, 128], BF16)
            nc.sync.dma_start(sel2[:], sel2_d)
            bqk = const.tile([128, 4], F32)
            nc.sync.dma_start(bqk[:], bqk_d)
            wv = wpool.tile([128, NDT, 256], BF16)
            nc.sync.dma_start(wv[:], wv_d.rearrange("(o p) f -> p o f", p=128))
            mask = const.tile([128, 896], BF16)
            nc.sync.dma_start(mask[:], mask_d)
            emit_xt_dma(1)
            wout = const.tile([128, 2, 1024], BF16)
            nc.sync.dma_start(wout[:], wout_d)
            emit_xt_dma(2)
            emit_xt_dma(3)

            def emit_fb(tt, fb):
                """One 128-col block of Q^T/K^T, bias fused into the evac."""
                xt = xts[tt]
                pq = ps.tile([128, TQ], F32, tag="pq", name=f"pq_{tt}_{fb}")
                for dt in range(NDT):
                    nc.tensor.matmul(pq[:], wqk[:, dt, fb * 128:(fb + 1) * 128],
                                     xt[:, dt, :],
                                     start=(dt == 0), stop=(dt == NDT - 1))
                nc.vector.tensor_scalar(
                    out=qkt[:, fb, tt * TQ:(tt + 1) * TQ], in0=pq[:],
                    scalar1=bqk[:, fb:fb + 1], scalar2=None, op0=ADD)

            def emit_v(tt, ts):
                """V rows for one 128-token tile (no bias: folded into bout)."""
                xt = xts[tt]
                pv = ps.tile([128, 512], F32, tag="pq", name=f"pv_{tt}_{ts}")
                for dt in range(NDT):
                    nc.tensor.matmul(pv[:, 0:256],
                                     xt[:, dt, ts * 128:(ts + 1) * 128],
                                     wv[:, dt, :], start=(dt == 0),
                                     stop=(dt == NDT - 1))
                nc.vector.tensor_copy(
                    vp[:, tt * 4 + ts, :, 0:64],
                    pv[:, 0:256].rearrange("p (h e) -> p h e", e=HD))

            # (PE-ns cost, closure) pairs for cost-weighted filler popping.
            # Q columns (fb 0,1) must exist before their block's attention;
            # K columns (fb 2,3) and V rows of block i are first consumed at
            # k-tile 4i, so they can fill inside their own block.
            def q_units(tt):
                return [(1704, (lambda tt=tt, fb=fb: emit_fb(tt, fb)))
                        for fb in range(2)]

            def k_units(tt):
                return [(1704, (lambda tt=tt, fb=fb: emit_fb(tt, fb)))
                        for fb in range(2, 4)]

            def v_units(tt):
                return [(852, (lambda tt=tt, ts=ts: emit_v(tt, ts)))
                        for ts in range(4)]

            def emit_scores(p, qi, kt, qlo=0, qhi=TQ):
                """QK^T for both heads of pair p + one paired exp (+ mask),
                restricted to block-local query columns [qlo, qhi).

                Score layout: sp[:, a, q] = scores of head a, keys on
                partitions, queries on free dim. Diagonal tiles only need
                columns >= 128*rr - qlo — bf16 matmuls run at 1 cycle/row at
                any moving size, so trim to the causal width.
                """
                w = qhi - qlo
                lo = 128 * (kt - 4 * qi) - qlo   # causal start, local cols
                c_lo = max(lo, 0)
                sp = ps.tile([128, 2, TQ], F32, tag="sp",
                             name=f"sp_{p}_{qi}_{kt}_{qlo}")
                for a in range(2):
                    rows = slice(64 * a, 64 * a + 64)
                    nc.tensor.matmul(
                        sp[:, a, c_lo:w], qkt[rows, 2 + p, kt * TK:(kt + 1) * TK],
                        qkt[rows, p,
                            qi * TQ + qlo + c_lo:qi * TQ + qhi],
                        start=True, stop=True)
                pt = pt_p.tile([128, 2, TQ], BF16, tag="pt",
                               name=f"pt_{p}_{qi}_{kt}_{qlo}")
                nc.scalar.activation(pt[:, :, c_lo:w], sp[:, :, c_lo:w], EXP)
                if -128 < lo < w:
                    c0 = 384 - 128 * (kt - 4 * qi) + qlo
                    m_lo, m_hi = c_lo, min(lo + 128, w)
                    nc.vector.tensor_tensor(
                        pt[:, :, m_lo:m_hi], pt[:, :, m_lo:m_hi],
                        mask[:, None, c0 + m_lo:c0 + m_hi].broadcast_to(
                            [128, 2, m_hi - m_lo]), MULT)
                return pt, c_lo

            def emit_c(p, qi, fillers, defer=True, qlo=0, qhi=TQ):
                """AV block for pair p over block-local query columns
                [qlo, qhi). Returns deferred normalize units."""
                w = qhi - qlo
                av0 = ps_av.tile([128, TQ], F32, tag="av0",
                                 name=f"av0_{p}_{qi}_{qlo}")
                av1 = ps_av.tile([128, TQ], F32, tag="av1",
                                 name=f"av1_{p}_{qi}_{qlo}")
                nkt = 4 * qi + qhi // TK    # causal: keys up to qi*TQ + qhi
                sc = {0: emit_scores(p, qi, 0, qlo, qhi)}
                # flush the previous pair's deferred normalize first (its
                # multiplies release the single-buffered av banks this pair's
                # first AV accumulation is about to claim), plus enough filler
                # to cover the first exp's latency
                for _ in range(min(2, len(fillers))):
                    fillers.popleft()[1]()
                for _k in (1, 2, 3, 4):
                    if _k < nkt:
                        sc[_k] = emit_scores(p, qi, _k, qlo, qhi)
                for kt in range(nkt):
                    # scores run several k-tiles ahead of the AV consuming
                    # them so each exp has slack, and filler work is popped by
                    # PE-time budget so the PE isn't gated on exp
                    if kt + 5 < nkt:
                        sc[kt + 5] = emit_scores(p, qi, kt + 5, qlo, qhi)
                    if fillers:
                        rem = sum(c for c, _ in fillers)
                        budget = rem / (nkt - kt)
                        popped = 0.0
                        while fillers and popped < budget:
                            c, f = fillers.popleft()
                            f()
                            popped += c
                    pt, c_lo = sc.pop(kt)
                    # av rows 0:64 = O^T, row 64 = softmax denominator
                    for a, av in ((0, av0), (1, av1)):
                        nc.tensor.matmul(
                            av[0:65, c_lo:w], vp[:, kt, 2 * p + a, :],
                            pt[:, a, c_lo:w],
                            start=(kt == 0), stop=(kt == nkt - 1),
                            skip_group_check=True)
                # Both reciprocals land on partition 0 of one tile so the
                # broadcast matmul's operands stay partition-aligned. The
                # normalize (pb broadcast + multiply) is deferred into the
                # next pair's filler stream to stay off the PE critical path;
                # the av banks free when the deferred multiplies drain.
                rec2 = nrm_p.tile([1, 2, TQ], BF16, tag="rec",
                                  name=f"rec_{p}_{qi}_{qlo}")
                nc.vector.reciprocal(rec2[0:1, 0, 0:w], av0[64:65, 0:w])
                nc.vector.reciprocal(rec2[0:1, 1, 0:w], av1[64:65, 0:w])

                def u_norm(a, av):
                    pb = ps.tile([64, TQ], F32, tag="sp",
                                 name=f"pb_{p}_{qi}_{a}_{qlo}")
                    nc.tensor.matmul(pb[0:64, 0:w], sel2[0:1, 0:64],
                                     rec2[0:1, a, 0:w],
                                     start=True, stop=True)
                    bc = nrm_p.tile([64, TQ], BF16, tag=f"bc{a}",
                                    name=f"bc_{p}_{qi}_{a}_{qlo}")
                    if defer:
                        nc.vector.tensor_copy(bc[0:64, 0:w], pb[0:64, 0:w])
                    else:
                        # tail: exp is done, Act takes the PSUM evacuation so
                        # the DVE chain (rec/mult) shortens
                        nc.scalar.activation(bc[0:64, 0:w], pb[0:64, 0:w], COPY)
                    nc.vector.tensor_tensor(
                        ot[64 * a:64 * a + 64, p,
                           qi * TQ + qlo:qi * TQ + qhi],
                        av[0:64, 0:w], bc[0:64, 0:w], MULT)

                units = [(213, lambda: u_norm(0, av0)),
                         (213, lambda: u_norm(1, av1))]
                if not defer:
                    for _, u in units:
                        u()
                    return []
                return units

            obs2 = {}

            def po_units(ts_list, act_ob=False):
                def emit_po(ts, dt):
                    # at the tail the score (sp) slots are idle: rotate the
                    # out-proj tiles through both pools for a deeper pipeline
                    po = ps.tile([128, 512], F32,
                                 tag=("sp" if act_ob and dt == 0 else "pq"),
                                 name=f"po_{ts}_{dt}")
                    for ft in range(2):
                        nc.tensor.matmul(
                            po[:], ot[:, ft, ts * 128:(ts + 1) * 128],
                            wout[:, ft, dt * 512:(dt + 1) * 512],
                            start=(ft == 0), stop=(ft == 1))
                    if act_ob:
                        # tail: one [128, 1024] DMA per token tile (HWDGE desc
                        # gen is the drain serializer), staged by the idle Act
                        # engine and issued alternately from the Act HWDGE and
                        # Pool SWDGE queues
                        if dt == 0:
                            obs2[ts] = nrm_p.tile([128, 2, 512], BF16,
                                                  tag="ob0", name=f"obt_{ts}")
                            nc.scalar.activation(obs2[ts][:, 0, :], po[:],
                                                 COPY)
                        else:
                            nc.vector.tensor_copy(obs2[ts][:, 1, :], po[:])
                            eng = nc.scalar if ts % 2 == 0 else nc.gpsimd
                            eng.dma_start(
                                out_ap[ts * 128:(ts + 1) * 128, :],
                                obs2[ts][:].rearrange("p a b -> p (a b)"))
                        return
                    ob = nrm_p.tile([128, 512], BF16, tag=f"ob{dt}",
                                    name=f"ob_{ts}_{dt}")
                    nc.vector.tensor_copy(ob[:], po[:])
                    nc.sync.dma_start(
                        out_ap[ts * 128:(ts + 1) * 128,
                               dt * 512:(dt + 1) * 512],
                        ob[:])
                return [
                    (426, (lambda ts=ts, dt=dt: emit_po(ts, dt)))
                    for ts in ts_list for dt in range(2)
                ]

            def po_fillers(qi):
                return po_units(range(4 * qi, 4 * qi + 4))

            # ---- Pipeline over 512-token query blocks ----
            # Block 0's Q/K columns: all four 128-col blocks accumulate in
            # parallel (pq banks + the av banks, idle until the first AV) so
            # the PE chews each wqk/xt0 chunk as it lands.
            pqs = [ps.tile([128, TQ], F32, tag="pq", name=f"pq0_{fb}")
                   for fb in range(2)]
            pqs += [ps_av.tile([128, TQ], F32, tag=f"av{fb - 2}",
                               name=f"pq0_{fb}") for fb in range(2, 4)]
            for dt in range(NDT):
                for fb in range(4):
                    nc.tensor.matmul(pqs[fb][:],
                                     wqk[:, dt, fb * 128:(fb + 1) * 128],
                                     xt0[:, dt, :],
                                     start=(dt == 0), stop=(dt == NDT - 1),
                                     skip_group_check=True)
            for fb in range(4):
                nc.vector.tensor_scalar(
                    out=qkt[:, fb, 0:TQ], in0=pqs[fb][:],
                    scalar1=bqk[:, fb:fb + 1], scalar2=None, op0=ADD)
            for ts in range(4):         # block 0's V rows
                emit_v(0, ts)
            carry = []                  # deferred normalize units
            # Per-block filler tranches, balanced against each block's
            # exp-side (Act) load: blocks 0/1 are PE-bound and need little;
            # blocks 2/3 are Act-bound and get their own K/V production plus
            # the deferred out-projections.
            tranche1 = {
                0: q_units(1) + k_units(1) + v_units(1),
                1: q_units(2) + k_units(2),
                2: v_units(2) + q_units(3) + po_fillers(0),
                3: k_units(3) + v_units(3) + po_fillers(1),
            }
            tranche2 = {3: po_fillers(2)}
            for tt in range(NQT):
                fl = deque()
                fl.extend(carry)
                carry = []
                fl.extend(tranche1[tt])
                n0 = emit_c(0, tt, fl)
                for u in reversed(n0):  # pair 0's normalize -> front of queue
                    fl.appendleft(u)
                fl.extend(tranche2.get(tt, []))
                carry = emit_c(1, tt, fl, defer=(tt < NQT - 1))
                while fl:
                    fl.popleft()[1]()
            for _, f in po_units(range(4 * (NQT - 1), 4 * NQT), act_ob=True):
                f()


_CACHE = {}


def _program():
    if "nc" in _CACHE:
        return _CACHE["nc"]
    nc = bacc.Bacc("TRN2", target_bir_lowering=False, debug=False)
    ins = {
        "xt": nc.dram_tensor("xt", [D, T], BF16, kind="ExternalInput").ap(),
        "wqk": nc.dram_tensor("wqk", [D, 512], BF16, kind="ExternalInput").ap(),
        "bqk": nc.dram_tensor("bqk", [128, 4], F32, kind="ExternalInput").ap(),
        "wv": nc.dram_tensor("wv", [D, 256], BF16, kind="ExternalInput").ap(),
        "wout": nc.dram_tensor("wout", [128, 2, 1024], BF16,
                               kind="ExternalInput").ap(),
        "mask": nc.dram_tensor("mask", [128, 896], BF16, kind="ExternalInput").ap(),
        "sel2": nc.dram_tensor("sel2", [2, 128], BF16, kind="ExternalInput").ap(),
    }
    out = nc.dram_tensor("out", [T, D], BF16, kind="ExternalOutput").ap()
    with tile.TileContext(nc) as tc:
        _build_mha(tc, out, ins)
    nc.compile()
    _CACHE["nc"] = nc
    return nc


def _in_maps(x, Wqkv, bqkv, Wout):
    bf16 = ml_dtypes.bfloat16
    x = np.asarray(x, dtype=np.float32)
    Wqkv = np.asarray(Wqkv, dtype=np.float32)
    bqkv = np.asarray(bqkv, dtype=np.float32)
    Wout = np.asarray(Wout, dtype=np.float32)
    scale = np.float32(1.0 / np.sqrt(HD))
    mask = (np.arange(128)[:, None] <= np.arange(896)[None, :] - 384).astype(bf16)
    sel2 = np.zeros((2, 128), dtype=bf16)
    sel2[0, 0:64] = 1.0
    sel2[1, 64:128] = 1.0
    maps = []
    for c in range(NCORES):
        b, hg = c // 4, c % 4
        hs = [4 * hg + i for i in range(HL)]
        q_cols = np.concatenate([Wqkv[:, h * HD:(h + 1) * HD] for h in hs], axis=1)
        k_cols = np.concatenate(
            [Wqkv[:, D + h * HD:D + (h + 1) * HD] for h in hs], axis=1)
        v_cols = np.concatenate(
            [Wqkv[:, 2 * D + h * HD:2 * D + (h + 1) * HD] for h in hs], axis=1)
        bq = np.concatenate([bqkv[h * HD:(h + 1) * HD] for h in hs])
        bk = np.concatenate([bqkv[D + h * HD:D + (h + 1) * HD] for h in hs])
        wqk = np.ascontiguousarray(
            np.concatenate([q_cols * scale, k_cols], axis=1)).astype(bf16)
        bqk = np.concatenate([bq * scale, bk]).reshape(4, 128).T
        wo = np.concatenate([Wout[h * HD:(h + 1) * HD, :] for h in hs], axis=0)
        wo = np.ascontiguousarray(
            wo.reshape(2, 128, D).transpose(1, 0, 2)).astype(bf16)
        maps.append({
            "xt": np.ascontiguousarray(x[b].T).astype(bf16),
            "wqk": wqk,
            "bqk": np.ascontiguousarray(bqk.astype(np.float32)),
            "wv": np.ascontiguousarray(v_cols).astype(bf16),
            "wout": wo,
            "mask": mask,
            "sel2": sel2,
        })
    return maps


def kernel(x, Wqkv, bqkv, Wout, bout):
    global LAST_RESULTS
    nc = _program()
    maps = _in_maps(x, Wqkv, bqkv, Wout)
    res = run_bass_kernel_spmd(nc, maps, list(range(NCORES)))
    LAST_RESULTS = res
    bout = np.asarray(bout, dtype=np.float32)
    bv_full = np.asarray(bqkv, dtype=np.float32)[2 * D:3 * D]
    bout_eff = bout + bv_full @ np.asarray(Wout, dtype=np.float32)
    out = np.empty((B, T, D), dtype=np.float32)
    for b in range(B):
        acc = res.results[4 * b]["out"].astype(np.float32)
        for hg in range(1, 4):
            acc = acc + res.results[4 * b + hg]["out"]
        out[b] = acc + bout_eff[None, :]
    return out


# revision 123
# speedup vs baseline: 1.0275x; 1.0275x over previous
"""Masked multi-head attention on 8 Trainium2 NeuronCores.

Sharding: core c = (b, hg) with b = c // 4, hg = c % 4. Each core computes the
full attention block for batch b restricted to heads [4*hg, 4*hg+4), including
its slice of the QKV projection and of the output projection. The host sums the
4 tensor-parallel partial outputs per batch and adds the output bias (which
also absorbs the V bias term: (O + bv) @ Wout = O @ Wout + bv @ Wout).

Shapes are hardcoded for B=2, T=2048, D=1024, H=16, Hd=64.

The data path is bf16 (x^T, weights, Q^T/K^T, P^T, V, O^T, out) with every
matmul accumulating in fp32 PSUM and the QK^T logits formed in fp32; only
input rounding is bf16. PE matmul throughput on TRN2 is identical for bf16
and fp32r, but bf16 halves DMA + SBUF and lifts the fp32r >=256 moving-dim
constraint, so diagonal score tiles trim to their true causal width.

Schedule highlights:
- host pre-transposes x, so the kernel runs zero PE transposes;
- Q/K bias fused into the PSUM evacuation (DVE tensor_scalar, per-partition);
- V bias folded into bout on the host;
- one paired exp per k-tile covers both heads of a pair (keys on partitions,
  queries on the free dim feed the AV matmul directly);
- softmax denominators ride a 65th ones-column in the V tiles; the normalize
  (reciprocal -> ones-broadcast matmul -> multiply) is deferred into the next
  pair's filler stream so it stays off the PE critical path, with its
  broadcast landing in the score (sp) PSUM slots to keep the pq slots free;
- scores run 5 k-tiles ahead of the AV consuming them, and independent PE
  work (next block's QKV, deferred out-projections) is popped from a
  cost-weighted filler queue so the PE is never gated on exp latency;
- filler tranches are balanced against each block's Act load: K columns and
  V rows of block i fill inside their own block (first consumed at k-tile
  4i), out-projections run two blocks late where QKV filler has run out;
- block 0's Q/K accumulates 4-ways in parallel (pq + the idle av banks)
  over dt-chunked wqk/x DMAs, behind a PE p-state warmup chain;
- the tail staggers output staging across Act + DVE and drains the last
  DMAs through both the Act HWDGE and Pool SWDGE queues in parallel.
"""

import numpy as np
from contextlib import ExitStack
from collections import deque

import ml_dtypes

import concourse.bass as bass
import concourse.bacc as bacc
import concourse.mybir as mybir
import concourse.tile as tile
from concourse.bass_utils import run_bass_kernel_spmd

B, T, D = 2, 2048, 1024
H, HD = 16, 64
HL = 4               # heads per core
NCORES = 8
TQ = 512             # query tile (matmul moving free dim)
TK = 128             # key tile
NQT = T // TQ        # 4
NKT = T // TK        # 16
NDT = D // 128       # 8

F32 = mybir.dt.float32
BF16 = mybir.dt.bfloat16
EXP = mybir.ActivationFunctionType.Exp
COPY = mybir.ActivationFunctionType.Copy
MULT = mybir.AluOpType.mult
ADD = mybir.AluOpType.add

LAST_RESULTS = None  # BassKernelResults of the most recent run (for test.py)


def _build_mha(tc, out_ap, in_aps):
    nc = tc.nc
    xt_d = in_aps["xt"]        # [D, T]  (x transposed on host, bf16)
    wqk_d = in_aps["wqk"]      # [D, 512]  (Qh0|Qh1|Qh2|Qh3|Kh0..Kh3, Q pre-scaled)
    bqk_d = in_aps["bqk"]      # [128, 4]  (col fb = per-partition bias, f32)
    wv_d = in_aps["wv"]        # [D, 256]
    wout_d = in_aps["wout"]    # [128, 2, 1024]
    mask_d = in_aps["mask"]    # [128, 896]

    with ExitStack() as ctx:
        ctx.enter_context(nc.allow_low_precision(reason="bf16 matmul pipeline"))
        const = ctx.enter_context(tc.tile_pool(name="const", bufs=1))
        big = ctx.enter_context(tc.tile_pool(name="big", bufs=1))

        # Persistent activations.
        # qkt[:, fb, t]: fb 0,1 = Q^T head pairs (0,1),(2,3); fb 2,3 = K^T pairs.
        # Rows 0:64 = even head of the pair, 64:128 = odd head.
        qkt = big.tile([128, 4, T], BF16)
        # vp[:, kt, h, 0:64] = V[kt*128:+128, h*64:+64]; vp[..., 64] = 1.0
        vp = big.tile([128, NKT, HL, 65], BF16)
        # ot[p][:, t]: normalized attention output^T; rows by head as in qkt.
        # One tile per head-pair so the out-projection's first (pair-0) matmul
        # doesn't falsely depend on the last pair-1 normalize write.
        ot = [big.tile([128, T], BF16, name=f"ot{p}") for p in range(2)]

        with ExitStack() as pctx:
            # PSUM: sp(2 banks x2) + pq(1 x2) + av0 + av1 = 8 banks
            ps = pctx.enter_context(tc.tile_pool(name="ps", bufs=2, space="PSUM"))
            ps_av = pctx.enter_context(tc.tile_pool(name="ps_av", bufs=1,
                                                    space="PSUM"))
            wpool = pctx.enter_context(tc.tile_pool(name="w", bufs=1))
            xt_p = pctx.enter_context(tc.tile_pool(name="xt", bufs=4))
            pt_p = pctx.enter_context(tc.tile_pool(name="ptile", bufs=4))
            nrm_p = pctx.enter_context(tc.tile_pool(name="nrm", bufs=2))

            # PE p-state warmup: harmless tiny matmuls keep the tensor engine
            # continuously busy through its ramp while the first DMAs land.
            # The operand comes from an on-device memset (no DMA round-trip).
            wsrc = const.tile([1, 64], BF16)
            nc.gpsimd.memset(wsrc[:], 1.0)
            warm = ps.tile([1, 64], F32, tag="pq")
            for _ in range(64):
                nc.tensor.matmul(warm[:], wsrc[0:1, 0:1], wsrc[:],
                                 start=True, stop=True)

            # softmax-denominator ones column of vp, built in-place
            nc.gpsimd.memset(vp[:, :, :, 64], 1.0)

            xts = {}

            def emit_xt_dma(tt):
                xt = xt_p.tile([128, NDT, TQ], BF16, tag="xt", name=f"xt_{tt}")
                src = xt_d[:, tt * TQ:(tt + 1) * TQ].rearrange(
                    "(o p) t -> p o t", p=128)
                nc.sync.dma_start(xt[:, 0:4, :], src[:, 0:4, :])
                nc.sync.dma_start(xt[:, 4:8, :], src[:, 4:8, :])
                xts[tt] = xt

            # DMA order = first-use order: the cost model's DMA engine pool
            # drains in issue order, so front-load what gates the pipeline.
            # wqk/xt0 stream in dt-pair chunks so the 4-way parallel QKV(0)
            # accumulation below can start on the first chunk.
            wqk = wpool.tile([128, NDT, 512], BF16)
            wqk_src = wqk_d.rearrange("(o p) f -> p o f", p=128)
            xt0 = xt_p.tile([128, NDT, TQ], BF16, tag="xt", name="xt_0")
            xt0_src = xt_d[:, 0:TQ].rearrange("(o p) t -> p o t", p=128)
            xts[0] = xt0
            for dh in range(4):
                s = slice(2 * dh, 2 * dh + 2)
                nc.sync.dma_start(wqk[:, s, :], wqk_src[:, s, :])
                nc.sync.dma_start(xt0[:, s, :], xt0_src[:, s, :])
            bqk = const.tile([128, 4], F32)
            nc.sync.dma_start(bqk[:], bqk_d)
            wv = wpool.tile([128, NDT, 256], BF16)
            nc.sync.dma_start(wv[:], wv_d.rearrange("(o p) f -> p o f", p=128))
            mask = const.tile([128, 896], BF16)
            nc.sync.dma_start(mask[:], mask_d)
            emit_xt_dma(1)
            wout = const.tile([128, 2, 1024], BF16)
            nc.sync.dma_start(wout[:], wout_d)
            emit_xt_dma(2)
            emit_xt_dma(3)

            def emit_fb(tt, fb):
                """One 128-col block of Q^T/K^T, bias fused into the evac."""
                xt = xts[tt]
                pq = ps.tile([128, TQ], F32, tag="pq", name=f"pq_{tt}_{fb}")
                for dt in range(NDT):
                    nc.tensor.matmul(pq[:], wqk[:, dt, fb * 128:(fb + 1) * 128],
                                     xt[:, dt, :],
                                     start=(dt == 0), stop=(dt == NDT - 1))
                nc.vector.tensor_scalar(
                    out=qkt[:, fb, tt * TQ:(tt + 1) * TQ], in0=pq[:],
                    scalar1=bqk[:, fb:fb + 1], scalar2=None, op0=ADD)

            def emit_v(tt, ts):
                """V rows for one 128-token tile (no bias: folded into bout)."""
                xt = xts[tt]
                pv = ps.tile([128, 512], F32, tag="pq", name=f"pv_{tt}_{ts}")
                for dt in range(NDT):
                    nc.tensor.matmul(pv[:, 0:256],
                                     xt[:, dt, ts * 128:(ts + 1) * 128],
                                     wv[:, dt, :], start=(dt == 0),
                                     stop=(dt == NDT - 1))
                nc.vector.tensor_copy(
                    vp[:, tt * 4 + ts, :, 0:64],
                    pv[:, 0:256].rearrange("p (h e) -> p h e", e=HD))

            # (PE-ns cost, closure) pairs for cost-weighted filler popping.
            # Q columns (fb 0,1) must exist before their block's attention;
            # K columns (fb 2,3) and V rows of block i are first consumed at
            # k-tile 4i, so they can fill inside their own block.
            def q_units(tt):
                return [(1704, (lambda tt=tt, fb=fb: emit_fb(tt, fb)))
                        for fb in range(2)]

            def k_units(tt):
                return [(1704, (lambda tt=tt, fb=fb: emit_fb(tt, fb)))
                        for fb in range(2, 4)]

            def v_units(tt):
                return [(852, (lambda tt=tt, ts=ts: emit_v(tt, ts)))
                        for ts in range(4)]

            def emit_scores(p, qi, kt, qlo=0, qhi=TQ):
                """QK^T for both heads of pair p + one paired exp (+ mask),
                restricted to block-local query columns [qlo, qhi).

                Score layout: sp[:, a, q] = scores of head a, keys on
                partitions, queries on free dim. Diagonal tiles only need
                columns >= 128*rr - qlo — bf16 matmuls run at 1 cycle/row at
                any moving size, so trim to the causal width.
                """
                w = qhi - qlo
                lo = 128 * (kt - 4 * qi) - qlo   # causal start, local cols
                c_lo = max(lo, 0)
                sp = ps.tile([128, 2, TQ], F32, tag="sp",
                             name=f"sp_{p}_{qi}_{kt}_{qlo}")
                for a in range(2):
                    rows = slice(64 * a, 64 * a + 64)
                    nc.tensor.matmul(
                        sp[:, a, c_lo:w], qkt[rows, 2 + p, kt * TK:(kt + 1) * TK],
                        qkt[rows, p,
                            qi * TQ + qlo + c_lo:qi * TQ + qhi],
                        start=True, stop=True)
                pt = pt_p.tile([128, 2, TQ], BF16, tag="pt",
                               name=f"pt_{p}_{qi}_{kt}_{qlo}")
                nc.scalar.activation(pt[:, :, c_lo:w], sp[:, :, c_lo:w], EXP)
                if -128 < lo < w:
                    c0 = 384 - 128 * (kt - 4 * qi) + qlo
                    m_lo, m_hi = c_lo, min(lo + 128, w)
                    nc.vector.tensor_tensor(
                        pt[:, :, m_lo:m_hi], pt[:, :, m_lo:m_hi],
                        mask[:, None, c0 + m_lo:c0 + m_hi].broadcast_to(
                            [128, 2, m_hi - m_lo]), MULT)
                return pt, c_lo

            def emit_c(p, qi, fillers, defer=True, qlo=0, qhi=TQ):
                """AV block for pair p over block-local query columns
                [qlo, qhi). Returns deferred normalize units."""
                w = qhi - qlo
                av0 = ps_av.tile([128, TQ], F32, tag="av0",
                                 name=f"av0_{p}_{qi}_{qlo}")
                av1 = ps_av.tile([128, TQ], F32, tag="av1",
                                 name=f"av1_{p}_{qi}_{qlo}")
                nkt = 4 * qi + qhi // TK    # causal: keys up to qi*TQ + qhi
                sc = {0: emit_scores(p, qi, 0, qlo, qhi)}
                # flush the previous pair's deferred normalize first (its
                # multiplies release the single-buffered av banks this pair's
                # first AV accumulation is about to claim), plus enough filler
                # to cover the first exp's latency
                for _ in range(min(2, len(fillers))):
                    fillers.popleft()[1]()
                for _k in (1, 2, 3, 4):
                    if _k < nkt:
                        sc[_k] = emit_scores(p, qi, _k, qlo, qhi)
                for kt in range(nkt):
                    # scores run several k-tiles ahead of the AV consuming
                    # them so each exp has slack, and filler work is popped by
                    # PE-time budget so the PE isn't gated on exp
                    if kt + 5 < nkt:
                        sc[kt + 5] = emit_scores(p, qi, kt + 5, qlo, qhi)
                    if fillers:
                        rem = sum(c for c, _ in fillers)
                        budget = rem / (nkt - kt)
                        popped = 0.0
                        while fillers and popped < budget:
                            c, f = fillers.popleft()
                            f()
                            popped += c
                    pt, c_lo = sc.pop(kt)
                    # av rows 0:64 = O^T, row 64 = softmax denominator
                    for a, av in ((0, av0), (1, av1)):
                        nc.tensor.matmul(
                            av[0:65, c_lo:w], vp[:, kt, 2 * p + a, :],
                            pt[:, a, c_lo:w],
                            start=(kt == 0), stop=(kt == nkt - 1),
                            skip_group_check=True)
                # Both reciprocals land on partition 0 of one tile so the
                # broadcast matmul's operands stay partition-aligned. The
                # normalize (pb broadcast + multiply) is deferred into the
                # next pair's filler stream to stay off the PE critical path;
                # the av banks free when the deferred multiplies drain.
                rec2 = nrm_p.tile([1, 2, TQ], BF16, tag="rec",
                                  name=f"rec_{p}_{qi}_{qlo}")
                nc.vector.reciprocal(rec2[0:1, 0, 0:w], av0[64:65, 0:w])
                nc.vector.reciprocal(rec2[0:1, 1, 0:w], av1[64:65, 0:w])

                def u_norm(a, av):
                    # replicate the reciprocal row across partitions with a
                    # Pool-issued DMA broadcast: no PE matmul, no DVE copy
                    bc = nrm_p.tile([64, TQ], BF16, tag=f"bc{a}",
                                    name=f"bc_{p}_{qi}_{a}_{qlo}")
                    nc.gpsimd.partition_broadcast(bc[0:64, 0:w],
                                                  rec2[0:1, a, 0:w])
                    nc.vector.tensor_tensor(
                        ot[p][64 * a:64 * a + 64,
                              qi * TQ + qlo:qi * TQ + qhi],
                        av[0:64, 0:w], bc[0:64, 0:w], MULT)

                units = [(50, lambda: u_norm(0, av0)),
                         (50, lambda: u_norm(1, av1))]
                if not defer:
                    for _, u in units:
                        u()
                    return []
                return units

            obs2 = {}

            def po_units(ts_list, act_ob=False):
                def emit_po(ts, dt):
                    # at the tail the score (sp) slots are idle: rotate the
                    # out-proj tiles through both pools for a deeper pipeline
                    po = ps.tile([128, 512], F32,
                                 tag=("sp" if act_ob and dt == 0 else "pq"),
                                 name=f"po_{ts}_{dt}")
                    for ft in range(2):
                        nc.tensor.matmul(
                            po[:], ot[ft][:, ts * 128:(ts + 1) * 128],
                            wout[:, ft, dt * 512:(dt + 1) * 512],
                            start=(ft == 0), stop=(ft == 1))
                    if act_ob:
                        # tail: one [128, 1024] DMA per token tile (HWDGE desc
                        # gen is the drain serializer), staged by the idle Act
                        # engine and issued alternately from the Act HWDGE and
                        # Pool SWDGE queues
                        if dt == 0:
                            obs2[ts] = nrm_p.tile([128, 2, 512], BF16,
                                                  tag="ob0", name=f"obt_{ts}")
                            nc.scalar.activation(obs2[ts][:, 0, :], po[:],
                                                 COPY)
                        else:
                            nc.vector.tensor_copy(obs2[ts][:, 1, :], po[:])
                            eng = nc.scalar if ts % 2 == 0 else nc.gpsimd
                            eng.dma_start(
                                out_ap[ts * 128:(ts + 1) * 128, :],
                                obs2[ts][:].rearrange("p a b -> p (a b)"))
                        return
                    ob = nrm_p.tile([128, 512], BF16, tag=f"ob{dt}",
                                    name=f"ob_{ts}_{dt}")
                    nc.vector.tensor_copy(ob[:], po[:])
                    nc.sync.dma_start(
                        out_ap[ts * 128:(ts + 1) * 128,
                               dt * 512:(dt + 1) * 512],
                        ob[:])
                return [
                    (426, (lambda ts=ts, dt=dt: emit_po(ts, dt)))
                    for ts in ts_list for dt in range(2)
                ]

            def po_fillers(qi):
                return po_units(range(4 * qi, 4 * qi + 4))

            # ---- Pipeline over 512-token query blocks ----
            # Block 0's Q/K columns: all four 128-col blocks accumulate in
            # parallel (pq banks + the av banks, idle until the first AV) so
            # the PE chews each wqk/xt0 chunk as it lands.
            pqs = [ps.tile([128, TQ], F32, tag="pq", name=f"pq0_{fb}")
                   for fb in range(2)]
            pqs += [ps_av.tile([128, TQ], F32, tag=f"av{fb - 2}",
                               name=f"pq0_{fb}") for fb in range(2, 4)]
            for dt in range(NDT):
                for fb in range(4):
                    nc.tensor.matmul(pqs[fb][:],
                                     wqk[:, dt, fb * 128:(fb + 1) * 128],
                                     xt0[:, dt, :],
                                     start=(dt == 0), stop=(dt == NDT - 1),
                                     skip_group_check=True)
            for fb in range(4):
                nc.vector.tensor_scalar(
                    out=qkt[:, fb, 0:TQ], in0=pqs[fb][:],
                    scalar1=bqk[:, fb:fb + 1], scalar2=None, op0=ADD)
            for ts in range(4):         # block 0's V rows
                emit_v(0, ts)
            carry = []                  # deferred normalize units
            # Per-block filler tranches, balanced against each block's
            # exp-side (Act) load: blocks 0/1 are PE-bound and need little;
            # blocks 2/3 are Act-bound and get their own K/V production plus
            # the deferred out-projections.
            tranche1 = {
                0: q_units(1) + k_units(1) + v_units(1),
                1: q_units(2) + k_units(2),
                2: v_units(2) + q_units(3) + po_fillers(0),
                3: k_units(3) + v_units(3) + po_fillers(1),
            }
            tranche2 = {3: po_fillers(2)}
            for tt in range(NQT):
                fl = deque()
                fl.extend(carry)
                carry = []
                fl.extend(tranche1[tt])
                n0 = emit_c(0, tt, fl)
                for u in reversed(n0):  # pair 0's normalize -> front of queue
                    fl.appendleft(u)
                fl.extend(tranche2.get(tt, []))
                carry = emit_c(1, tt, fl, defer=(tt < NQT - 1))
                while fl:
                    fl.popleft()[1]()
            for _, f in po_units(range(4 * (NQT - 1), 4 * NQT), act_ob=True):
                f()


_CACHE = {}


def _program():
    if "nc" in _CACHE:
        return _CACHE["nc"]
    nc = bacc.Bacc("TRN2", target_bir_lowering=False, debug=False)
    ins = {
        "xt": nc.dram_tensor("xt", [D, T], BF16, kind="ExternalInput").ap(),
        "wqk": nc.dram_tensor("wqk", [D, 512], BF16, kind="ExternalInput").ap(),
        "bqk": nc.dram_tensor("bqk", [128, 4], F32, kind="ExternalInput").ap(),
        "wv": nc.dram_tensor("wv", [D, 256], BF16, kind="ExternalInput").ap(),
        "wout": nc.dram_tensor("wout", [128, 2, 1024], BF16,
                               kind="ExternalInput").ap(),
        "mask": nc.dram_tensor("mask", [128, 896], BF16, kind="ExternalInput").ap(),
    }
    out = nc.dram_tensor("out", [T, D], BF16, kind="ExternalOutput").ap()
    with tile.TileContext(nc) as tc:
        _build_mha(tc, out, ins)
    nc.compile()
    _CACHE["nc"] = nc
    return nc


def _in_maps(x, Wqkv, bqkv, Wout):
    bf16 = ml_dtypes.bfloat16
    x = np.asarray(x, dtype=np.float32)
    Wqkv = np.asarray(Wqkv, dtype=np.float32)
    bqkv = np.asarray(bqkv, dtype=np.float32)
    Wout = np.asarray(Wout, dtype=np.float32)
    scale = np.float32(1.0 / np.sqrt(HD))
    mask = (np.arange(128)[:, None] <= np.arange(896)[None, :] - 384).astype(bf16)
    maps = []
    for c in range(NCORES):
        b, hg = c // 4, c % 4
        hs = [4 * hg + i for i in range(HL)]
        q_cols = np.concatenate([Wqkv[:, h * HD:(h + 1) * HD] for h in hs], axis=1)
        k_cols = np.concatenate(
            [Wqkv[:, D + h * HD:D + (h + 1) * HD] for h in hs], axis=1)
        v_cols = np.concatenate(
            [Wqkv[:, 2 * D + h * HD:2 * D + (h + 1) * HD] for h in hs], axis=1)
        bq = np.concatenate([bqkv[h * HD:(h + 1) * HD] for h in hs])
        bk = np.concatenate([bqkv[D + h * HD:D + (h + 1) * HD] for h in hs])
        wqk = np.ascontiguousarray(
            np.concatenate([q_cols * scale, k_cols], axis=1)).astype(bf16)
        bqk = np.concatenate([bq * scale, bk]).reshape(4, 128).T
        wo = np.concatenate([Wout[h * HD:(h + 1) * HD, :] for h in hs], axis=0)
        wo = np.ascontiguousarray(
            wo.reshape(2, 128, D).transpose(1, 0, 2)).astype(bf16)
        maps.append({
            "xt": np.ascontiguousarray(x[b].T).astype(bf16),
            "wqk": wqk,
            "bqk": np.ascontiguousarray(bqk.astype(np.float32)),
            "wv": np.ascontiguousarray(v_cols).astype(bf16),
            "wout": wo,
            "mask": mask,
        })
    return maps


def kernel(x, Wqkv, bqkv, Wout, bout):
    global LAST_RESULTS
    nc = _program()
    maps = _in_maps(x, Wqkv, bqkv, Wout)
    res = run_bass_kernel_spmd(nc, maps, list(range(NCORES)))
    LAST_RESULTS = res
    bout = np.asarray(bout, dtype=np.float32)
    bv_full = np.asarray(bqkv, dtype=np.float32)[2 * D:3 * D]
    bout_eff = bout + bv_full @ np.asarray(Wout, dtype=np.float32)
    out = np.empty((B, T, D), dtype=np.float32)
    for b in range(B):
        acc = res.results[4 * b]["out"].astype(np.float32)
        for hg in range(1, 4):
            acc = acc + res.results[4 * b + hg]["out"]
        out[b] = acc + bout_eff[None, :]
    return out
